# revision 1
# baseline (speedup 1.0000x reference)
"""Multi-head attention (B=4, S=2048, D=1024, H=16, Dh=64) on 8 TRN2 NeuronCores.

Sharding: core = (batch, head_group) with 4 batches x 2 head-groups of 8 heads.
Fully data-parallel SPMD - no collectives. Each core computes
out[b, :, hg*512:(hg+1)*512].

Per-core kernel (all fp32 storage, float32r matmuls):
  phase 1: project K^T, V, Q^T from host-transposed seq inputs
           (d_model on partitions, contracted by the PE).
  phase 2: for each head-quad and q-block of 512:
           scores S^T[k,q] per k-tile via row-paired K=64 matmuls,
           exp on ScalarE (scale=1/8, per-partition bias = -1e6 at the
           masked key position), AV via col-paired M=64 matmuls
           accumulating O^T in PSUM, softmax denominators via M=1
           ones-matmuls col-tiled 4-way, then PE-transpose O^T and the
           sums back to q-major and normalize on VectorE.
"""

from contextlib import ExitStack

import numpy as np

import concourse.bass as bass
import concourse.bacc as bacc
import concourse.mybir as mybir
import concourse.tile as tile
from concourse.bass_utils import run_bass_kernel_spmd
from concourse.masks import make_identity

B = 4
SEQ = 2048
DM = 1024
H = 16
DH = 64
NCORES = 8
CPC = 512          # output columns per core (8 heads x 64)
P = 128
NQB = SEQ // 512   # q blocks of 512
NKT = SEQ // P     # k tiles of 128
NDT = DM // P      # d_model tiles of 128

F32 = mybir.dt.float32
F32R = mybir.dt.float32r
BF16 = mybir.dt.bfloat16
EXP = mybir.ActivationFunctionType.Exp

_compiled = None


def _r(ap):
    return ap.bitcast(F32R)


def _emit(ctx: ExitStack, tc: tile.TileContext, qt, kt, vt, wq, wk, wv, bmask, out, seq=SEQ):
    nc = tc.nc
    NQB = seq // 512
    NKT = seq // P

    proj = ctx.enter_context(tc.tile_pool(name="proj", bufs=1))
    small = ctx.enter_context(tc.tile_pool(name="small", bufs=1))
    stage = ctx.enter_context(tc.tile_pool(name="stage", bufs=2))
    wpool = ctx.enter_context(tc.tile_pool(name="wpool", bufs=1))
    epool = ctx.enter_context(tc.tile_pool(name="epool", bufs=6))
    opool = ctx.enter_context(tc.tile_pool(name="opool", bufs=2))
    oparts = ctx.enter_context(tc.tile_pool(name="oparts", bufs=1))
    ps_sc = ctx.enter_context(tc.tile_pool(name="ps_sc", bufs=2, space="PSUM"))
    ps_ot = ctx.enter_context(tc.tile_pool(name="ps_ot", bufs=2, space="PSUM"))
    ps_sm = ctx.enter_context(tc.tile_pool(name="ps_sm", bufs=1, space="PSUM"))
    ps_tr = ctx.enter_context(tc.tile_pool(name="ps_tr", bufs=1, space="PSUM"))

    ident = small.tile([P, P], F32)
    make_identity(nc, ident[:])
    ones_f = small.tile([P, 1], F32)
    nc.vector.memset(ones_f[:], 1.0)
    ones = small.tile([P, 1], BF16)
    nc.vector.tensor_copy(ones[:], ones_f[:])
    bmask_sb = small.tile([P, NKT], F32)
    nc.sync.dma_start(bmask_sb[:], bmask.ap())

    w_sb = {}
    for name, w in (("wq", wq), ("wk", wk), ("wv", wv)):
        w_sb[name] = wpool.tile([P, NDT, CPC], F32R, tag=name, name=name)
        w_r = w.ap().rearrange("(dt p) c -> p dt c", p=P)
        for dt in range(NDT):
            raw = stage.tile([P, 1, 512], F32, tag="raw")
            nc.sync.dma_start(raw[:], w_r[:, dt:dt + 1, :])
            nc.vector.tensor_copy(w_sb[name][:, dt:dt + 1, :], raw[:])

    kproj = [proj.tile([P, seq], F32R, tag=f"kproj{p}", name=f"kproj{p}") for p in range(4)]
    qproj = [proj.tile([P, seq], F32R, tag=f"qproj{p}", name=f"qproj{p}") for p in range(4)]
    v_sb = proj.tile([P, NKT, 512], BF16, tag="v_sb")

    def stage_block(src_r, blk):
        st = stage.tile([P, NDT, 512], F32R, tag="stage")
        for dt in range(NDT):
            raw = stage.tile([P, 1, 512], F32, tag="raw")
            nc.sync.dma_start(raw[:], src_r[:, dt:dt + 1, blk * 512:(blk + 1) * 512])
            nc.vector.tensor_copy(st[:, dt:dt + 1, :], raw[:])
        return st

    def proj_qk(name, st, dsts, qb):
        for p in range(4):
            ps = ps_sc.tile([P, 1024], F32, tag="scores")
            for dt in range(NDT):
                nc.tensor.matmul(
                    ps[:, 0:512],
                    w_sb[name][:, dt, 128 * p:128 * (p + 1)],
                    st[:, dt, :],
                    start=(dt == 0),
                    stop=(dt == NDT - 1),
                )
            nc.vector.tensor_copy(dsts[p][:, qb * 512:(qb + 1) * 512], ps[:, 0:512])

    # ---- phase 1: K and V projections (full) ----------------------------
    kt_r = kt.ap().rearrange("(dt p) q -> p dt q", p=P)
    vt_r = vt.ap().rearrange("(dt p) q -> p dt q", p=P)
    qt_r = qt.ap().rearrange("(dt p) q -> p dt q", p=P)
    for kb in range(NQB):
        st = stage_block(kt_r, kb)
        proj_qk("wk", st, kproj, kb)
        st = stage_block(vt_r, kb)
        for sub in range(4):
            kt_i = kb * 4 + sub
            ps = ps_sc.tile([P, 1024], F32, tag="scores")
            for dt in range(NDT):
                nc.tensor.matmul(
                    ps[:, 0:512],
                    st[:, dt, 128 * sub:128 * (sub + 1)],
                    w_sb["wv"][:, dt, :],
                    start=(dt == 0),
                    stop=(dt == NDT - 1),
                )
            nc.vector.tensor_copy(v_sb[:, kt_i, :], ps[:, 0:512])

    # ---- phase 2: per q-block: project Q, then attention ---------------
    for qb in range(NQB):
        st = stage_block(qt_r, qb)
        proj_qk("wq", st, qproj, qb)
        qs = slice(qb * 512, (qb + 1) * 512)
        for quad in range(2):
            pairs = (2 * quad, 2 * quad + 1)
            ot_ps = [ps_ot.tile([P, 512], F32, tag="ot", name=f"ot{i}") for i in range(2)]
            sm_ps = ps_sm.tile([P, 512], F32, tag="sums")
            for kt_i in range(NKT):
                e_tiles = []
                for pi, pr in enumerate(pairs):
                    st_ps = ps_sc.tile([P, 1024], F32, tag="scores")
                    for hh in range(2):
                        rows = slice(64 * hh, 64 * (hh + 1))
                        nc.tensor.matmul(
                            st_ps[:, 512 * hh:512 * (hh + 1)],
                            kproj[pr][rows, kt_i * P:(kt_i + 1) * P],
                            qproj[pr][rows, qs],
                            start=True,
                            stop=True,
                            tile_position=(64 * hh, 0),
                        )
                    e = epool.tile([P, 1024], BF16, tag="e")
                    nc.scalar.activation(
                        e[:], st_ps[:], EXP,
                        bias=bmask_sb[:, kt_i:kt_i + 1], scale=0.125,
                    )
                    e_tiles.append(e)
                    for hh in range(2):
                        cols = slice(128 * pr + 64 * hh, 128 * pr + 64 * (hh + 1))
                        nc.tensor.matmul(
                            ot_ps[pi][64 * hh:64 * (hh + 1), :],
                            v_sb[:, kt_i, cols],
                            e[:, 512 * hh:512 * (hh + 1)],
                            start=(kt_i == 0),
                            stop=(kt_i == NKT - 1),
                            tile_position=(0, 64 * hh),
                            skip_group_check=(hh == 1),
                        )
                for j in range(4):
                    nc.tensor.matmul(
                        sm_ps[32 * j:32 * j + 1, :],
                        ones[:],
                        e_tiles[j // 2][:, 512 * (j % 2):512 * (j % 2 + 1)],
                        start=(kt_i == 0),
                        stop=(kt_i == NKT - 1),
                        tile_position=(0, 32 * j),
                        skip_group_check=(j > 0),
                    )

            # ---- tail: transpose + normalize + store -------------------
            sums_sb = opool.tile([P, 512], F32, tag="sums_sb")
            nc.vector.memset(sums_sb[:], 1.0)
            for j in range(4):
                nc.vector.tensor_copy(
                    sums_sb[32 * j:32 * j + 1, :], sm_ps[32 * j:32 * j + 1, :]
                )
            rcp = opool.tile([P, 16], F32, tag="rcp")
            for c in range(4):
                tr_s = ps_tr.tile([P, P], F32, tag="trp")
                nc.tensor.transpose(tr_s[:], sums_sb[:, c * P:(c + 1) * P], ident[:])
                for j in range(4):
                    nc.vector.reciprocal(
                        rcp[:, 4 * c + j:4 * c + j + 1], tr_s[:, 32 * j:32 * j + 1]
                    )
            o_part = oparts.tile([P, 4, 256], F32, tag="opart")
            for pi in range(2):
                ot_sb = opool.tile([P, 512], F32, tag="ot_sb")
                nc.vector.tensor_copy(ot_sb[:], ot_ps[pi][:])
                for c in range(4):
                    tr_o = ps_tr.tile([P, P], F32, tag="trp")
                    nc.tensor.transpose(tr_o[:], ot_sb[:, c * P:(c + 1) * P], ident[:])
                    for hh in range(2):
                        lh = 2 * pi + hh
                        nc.vector.tensor_scalar(
                            o_part[:, c, 64 * lh:64 * (lh + 1)],
                            tr_o[:, 64 * hh:64 * (hh + 1)],
                            rcp[:, 4 * c + lh:4 * c + lh + 1],
                            None,
                            mybir.AluOpType.mult,
                        )
            for c in range(4):
                nc.sync.dma_start(
                    out.ap()[
                        qb * 512 + c * P:qb * 512 + (c + 1) * P,
                        quad * 256:(quad + 1) * 256,
                    ],
                    o_part[:, c, :],
                )


def build(seq=SEQ):
    global _compiled
    if seq == SEQ and _compiled is not None:
        return _compiled
    nc = bacc.Bacc("TRN2", target_bir_lowering=False, debug=False)
    qt = nc.dram_tensor("qt", [DM, seq], F32, kind="ExternalInput")
    kt = nc.dram_tensor("kt", [DM, seq], F32, kind="ExternalInput")
    vt = nc.dram_tensor("vt", [DM, seq], F32, kind="ExternalInput")
    wq = nc.dram_tensor("wq", [DM, CPC], F32, kind="ExternalInput")
    wk = nc.dram_tensor("wk", [DM, CPC], F32, kind="ExternalInput")
    wv = nc.dram_tensor("wv", [DM, CPC], F32, kind="ExternalInput")
    bmask = nc.dram_tensor("bmask", [P, seq // P], F32, kind="ExternalInput")
    out = nc.dram_tensor("out", [seq, CPC], F32, kind="ExternalOutput")
    with tile.TileContext(nc) as tc:
        with ExitStack() as ctx:
            _emit(ctx, tc, qt, kt, vt, wq, wk, wv, bmask, out, seq=seq)
    nc.compile()
    if seq == SEQ:
        _compiled = nc
    return nc


def make_in_maps(Q_seq, K_seq, V_seq, V_len, WQ, WK, WV):
    in_maps = []
    for core in range(NCORES):
        b, hg = divmod(core, 2)
        cols = slice(hg * CPC, (hg + 1) * CPC)
        bm = np.zeros((P, NKT), np.float32)
        vl = int(V_len[b, 0])
        bm[vl % P, vl // P] = -1e6
        in_maps.append(
            {
                "qt": np.ascontiguousarray(Q_seq[b].T),
                "kt": np.ascontiguousarray(K_seq[b].T),
                "vt": np.ascontiguousarray(V_seq[b].T),
                "wq": np.ascontiguousarray(WQ[:, cols]),
                "wk": np.ascontiguousarray(WK[:, cols]),
                "wv": np.ascontiguousarray(WV[:, cols]),
                "bmask": bm,
            }
        )
    return in_maps


def kernel(Q_seq, K_seq, V_seq, Q_len, V_len, WQ, WK, WV, _trace=False):
    nc = build()
    in_maps = make_in_maps(Q_seq, K_seq, V_seq, V_len, WQ, WK, WV)
    res = run_bass_kernel_spmd(
        nc, in_maps, core_ids=list(range(NCORES)), trace=_trace
    )
    out = np.empty((B, SEQ, H * DH), np.float32)
    for core in range(NCORES):
        b, hg = divmod(core, 2)
        out[b, :, hg * CPC:(hg + 1) * CPC] = res.results[core]["out"]
    for b in range(B):
        out[b, int(Q_len[b, 0]), :] = 0.0
    if _trace:
        kernel._last_results = res
    return out



# revision 4
# speedup vs baseline: 2.5584x; 2.5584x over previous
"""Multi-head attention (B=4, S=2048, D=1024, H=16, Dh=64) on 8 TRN2 NeuronCores.

Sharding: core = (batch, head_group) with 4 batches x 2 head-groups of 8 heads.
Fully data-parallel SPMD - no collectives. Each core computes
out[b, :, hg*512:(hg+1)*512].

v2: all matmul operands in bf16 (inputs cast host-side, halving DMA and
PE passes vs fp32r), and the softmax denominators come for free from a
ones-column fused into each head's V stationary ([128, 65] per head ->
output row 64 accumulates sum(exp)).

Per-core kernel:
  phase 1: project K^T (d-major), V (k-major + ones col), Q^T.
  phase 2: per q-block of 512 and head-quad of 4: scores S^T[k,q] per
           k-tile via row-paired Dh=64 matmuls, exp on ScalarE
           (scale=1/8, per-partition bias -1e6 at the masked key
           position), AV via [128,65] stationaries accumulating
           [O^T; sums] in per-head PSUM tiles, then PE-transpose back
           to q-major and normalize on VectorE.
"""

from contextlib import ExitStack

import ml_dtypes
import numpy as np

import concourse.bass as bass
import concourse.bacc as bacc
import concourse.mybir as mybir
import concourse.tile as tile
from concourse.bass_utils import run_bass_kernel_spmd
from concourse.masks import make_identity

B = 4
SEQ = 2048
DM = 1024
H = 16
DH = 64
NCORES = 8
CPC = 512          # output columns per core (8 heads x 64)
P = 128
NQB = SEQ // 512   # q blocks of 512
NKT = SEQ // P     # k tiles of 128
NDT = DM // P      # d_model tiles of 128

F32 = mybir.dt.float32
BF16 = mybir.dt.bfloat16
EXP = mybir.ActivationFunctionType.Exp

_compiled = None


def _emit(ctx: ExitStack, tc: tile.TileContext, qt, kt, vt, wq, wk, wv, bmask, out):
    nc = tc.nc

    small = ctx.enter_context(tc.tile_pool(name="small", bufs=1))
    in_sb = ctx.enter_context(tc.tile_pool(name="in_sb", bufs=1))
    proj = ctx.enter_context(tc.tile_pool(name="proj", bufs=1))
    wpool = ctx.enter_context(tc.tile_pool(name="wpool", bufs=1))
    epool = ctx.enter_context(tc.tile_pool(name="epool", bufs=6))
    opool = ctx.enter_context(tc.tile_pool(name="opool", bufs=2))
    rpool = ctx.enter_context(tc.tile_pool(name="rpool", bufs=2))
    oparts = ctx.enter_context(tc.tile_pool(name="oparts", bufs=2))
    ps_sc = ctx.enter_context(tc.tile_pool(name="ps_sc", bufs=2, space="PSUM"))
    ps_ot = ctx.enter_context(tc.tile_pool(name="ps_ot", bufs=4, space="PSUM"))

    ident = small.tile([P, P], F32)
    make_identity(nc, ident[:])
    bmask_sb = small.tile([P, NKT], F32)
    nc.sync.dma_start(bmask_sb[:], bmask.ap())

    w_sb = {}
    for name, w in (("wk", wk), ("wv", wv), ("wq", wq)):
        w_sb[name] = wpool.tile([P, NDT, CPC], BF16, tag=name, name=name)

    kt_sb = in_sb.tile([P, NDT, SEQ], BF16, tag="kt_sb")
    vt_sb = in_sb.tile([P, NDT, SEQ], BF16, tag="vt_sb")
    qt_sb = in_sb.tile([P, NDT, SEQ], BF16, tag="qt_sb")

    # DMA issue order follows first-use order: weights, then per-512-block
    # K, V, Q column slices.
    nc.sync.dma_start(w_sb["wk"][:], wk.ap().rearrange("(dt p) c -> p dt c", p=P))
    nc.sync.dma_start(w_sb["wv"][:], wv.ap().rearrange("(dt p) c -> p dt c", p=P))
    nc.sync.dma_start(w_sb["wq"][:], wq.ap().rearrange("(dt p) c -> p dt c", p=P))
    for blk in range(NQB):
        cs = slice(blk * 512, (blk + 1) * 512)
        for t, t_sb in ((kt, kt_sb), (vt, vt_sb), (qt, qt_sb)):
            nc.sync.dma_start(
                t_sb[:, :, cs],
                t.ap().rearrange("(dt p) q -> p dt q", p=P)[:, :, cs],
            )

    kproj = [proj.tile([P, SEQ], BF16, tag=f"kproj{p}", name=f"kproj{p}") for p in range(4)]
    qproj = [proj.tile([P, SEQ], BF16, tag=f"qproj{p}", name=f"qproj{p}") for p in range(4)]
    v_ones = proj.tile([P, NKT, 8, 65], BF16, tag="v_ones")
    nc.vector.memset(v_ones[:], 1.0)

    def proj_kq(name, src_sb, dsts, blk):
        cs = slice(blk * 512, (blk + 1) * 512)
        for p in range(4):
            ps = ps_sc.tile([P, 1024], F32, tag="scores")
            for dt in range(NDT):
                nc.tensor.matmul(
                    ps[:, 0:512],
                    w_sb[name][:, dt, 128 * p:128 * (p + 1)],
                    src_sb[:, dt, cs],
                    start=(dt == 0),
                    stop=(dt == NDT - 1),
                )
            nc.vector.tensor_copy(dsts[p][:, cs], ps[:, 0:512])

    def proj_v(kb):
        for sub in range(4):
            kt_i = kb * 4 + sub
            ps = ps_sc.tile([P, 1024], F32, tag="scores")
            for dt in range(NDT):
                nc.tensor.matmul(
                    ps[:, 0:512],
                    vt_sb[:, dt, kt_i * P:(kt_i + 1) * P],
                    w_sb["wv"][:, dt, :],
                    start=(dt == 0),
                    stop=(dt == NDT - 1),
                )
            nc.vector.tensor_copy(
                v_ones[:, kt_i, :, 0:64],
                ps[:, 0:512].rearrange("p (h c) -> p h c", c=64),
            )

    # ---- phase 1: projections -----------------------------------------
    for kb in range(NQB):
        proj_kq("wk", kt_sb, kproj, kb)
        proj_v(kb)
    proj_kq("wq", qt_sb, qproj, 0)

    # ---- phase 2: attention -------------------------------------------
    for qb in range(NQB):
        if qb > 0:
            proj_kq("wq", qt_sb, qproj, qb)
        qs = slice(qb * 512, (qb + 1) * 512)
        for quad in range(2):
            ot = [ps_ot.tile([P, 512], F32, tag="ot", name=f"ot{quad}_{i}")
                  for i in range(4)]
            for kt_i in range(NKT):
                for pi in range(2):
                    pr = 2 * quad + pi
                    sps = ps_sc.tile([P, 1024], F32, tag="scores")
                    for hh in range(2):
                        rows = slice(64 * hh, 64 * (hh + 1))
                        nc.tensor.matmul(
                            sps[:, 512 * hh:512 * (hh + 1)],
                            kproj[pr][rows, kt_i * P:(kt_i + 1) * P],
                            qproj[pr][rows, qs],
                            start=True,
                            stop=True,
                            tile_position=(64 * hh, 0),
                        )
                    e = epool.tile([P, 1024], BF16, tag="e")
                    nc.scalar.activation(
                        e[:], sps[:], EXP,
                        bias=bmask_sb[:, kt_i:kt_i + 1], scale=0.125,
                    )
                    for hh in range(2):
                        h = 4 * quad + 2 * pi + hh
                        nc.tensor.matmul(
                            ot[2 * pi + hh][0:65, :],
                            v_ones[:, kt_i, h, :],
                            e[:, 512 * hh:512 * (hh + 1)],
                            start=(kt_i == 0),
                            stop=(kt_i == NKT - 1),
                        )

            # ---- tail: transpose + normalize + store -------------------
            o_part = oparts.tile([P, 4, 256], F32, tag="opart")
            for i in range(4):
                ot_sb = opool.tile([P, 512], F32, tag="ot_sb")
                nc.vector.tensor_copy(ot_sb[0:65, :], ot[i][0:65, :])
                tr = ps_ot.tile([P, 512], F32, tag="ot", name=f"tr{quad}_{i}")
                rcp = rpool.tile([P, 4], F32, tag="rcp")
                for c in range(4):
                    nc.tensor.transpose(
                        tr[:, 65 * c:65 * (c + 1)],
                        ot_sb[0:65, c * P:(c + 1) * P],
                        ident[0:65, 0:65],
                    )
                for c in range(4):
                    nc.vector.reciprocal(
                        rcp[:, c:c + 1], tr[:, 65 * c + 64:65 * c + 65]
                    )
                for c in range(4):
                    nc.vector.tensor_scalar(
                        o_part[:, c, 64 * i:64 * (i + 1)],
                        tr[:, 65 * c:65 * c + 64],
                        rcp[:, c:c + 1],
                        None,
                        mybir.AluOpType.mult,
                    )
            for c in range(4):
                nc.sync.dma_start(
                    out.ap()[
                        qb * 512 + c * P:qb * 512 + (c + 1) * P,
                        quad * 256:(quad + 1) * 256,
                    ],
                    o_part[:, c, :],
                )


def build():
    global _compiled
    if _compiled is not None:
        return _compiled
    nc = bacc.Bacc("TRN2", target_bir_lowering=False, debug=False)
    qt = nc.dram_tensor("qt", [DM, SEQ], BF16, kind="ExternalInput")
    kt = nc.dram_tensor("kt", [DM, SEQ], BF16, kind="ExternalInput")
    vt = nc.dram_tensor("vt", [DM, SEQ], BF16, kind="ExternalInput")
    wq = nc.dram_tensor("wq", [DM, CPC], BF16, kind="ExternalInput")
    wk = nc.dram_tensor("wk", [DM, CPC], BF16, kind="ExternalInput")
    wv = nc.dram_tensor("wv", [DM, CPC], BF16, kind="ExternalInput")
    bmask = nc.dram_tensor("bmask", [P, NKT], F32, kind="ExternalInput")
    out = nc.dram_tensor("out", [SEQ, CPC], F32, kind="ExternalOutput")
    with tile.TileContext(nc) as tc:
        with ExitStack() as ctx:
            _emit(ctx, tc, qt, kt, vt, wq, wk, wv, bmask, out)
    nc.compile()
    _compiled = nc
    return nc


def make_in_maps(Q_seq, K_seq, V_seq, V_len, WQ, WK, WV):
    bf = ml_dtypes.bfloat16
    in_maps = []
    qkv_t = {}
    for b in range(B):
        qkv_t[b] = tuple(
            np.ascontiguousarray(x[b].T).astype(bf) for x in (Q_seq, K_seq, V_seq)
        )
    w_bf = {hg: tuple(
        np.ascontiguousarray(w[:, hg * CPC:(hg + 1) * CPC]).astype(bf)
        for w in (WQ, WK, WV)) for hg in range(2)}
    for core in range(NCORES):
        b, hg = divmod(core, 2)
        bm = np.zeros((P, NKT), np.float32)
        vl = int(V_len[b, 0])
        bm[vl % P, vl // P] = -1e6
        qt, kt, vt = qkv_t[b]
        wq, wk, wv = w_bf[hg]
        in_maps.append(
            {"qt": qt, "kt": kt, "vt": vt, "wq": wq, "wk": wk, "wv": wv,
             "bmask": bm}
        )
    return in_maps


def kernel(Q_seq, K_seq, V_seq, Q_len, V_len, WQ, WK, WV, _trace=False):
    nc = build()
    in_maps = make_in_maps(Q_seq, K_seq, V_seq, V_len, WQ, WK, WV)
    res = run_bass_kernel_spmd(
        nc, in_maps, core_ids=list(range(NCORES)), trace=_trace
    )
    out = np.empty((B, SEQ, H * DH), np.float32)
    for core in range(NCORES):
        b, hg = divmod(core, 2)
        out[b, :, hg * CPC:(hg + 1) * CPC] = res.results[core]["out"]
    for b in range(B):
        out[b, int(Q_len[b, 0]), :] = 0.0
    if _trace:
        kernel._last_results = res
    return out
